# revision 21
# baseline (speedup 1.0000x reference)
"""Trainium2 Bass kernel for nn_DGASEncoder (PointNet++-style ball-query encoder).

Self-contained: hardcoded shapes; takes FULL inputs, shards across 8 NeuronCores
(data-parallel over (batch, N/2)), returns the FULL output.

Per-core pipeline (SPMD; cores differ only in input data), stage-major emission
so independent query-tiles pipeline across engines:
  A) conv1d+BN stats: h = W1 @ f over all B*N via PE; bn_stats/bn_aggr -> mean/var;
     h_n = relu(gamma'*h_b + beta') for this core's batch.
  B) ball query: squared distances via a 24-row bf16-split matmul (f32-exact dot);
     v = saturated sigmoid in {0,1} on ACT; per-chunk ranks via one fp-exact
     triangular matmul per chunk; per-chunk counts via 1-col matmuls into one
     PSUM bank; batched global prefix on DVE; slot ids evacuated on GpSimd with
     the cross-chunk carry fused in (PSUM + CT -> int16).
  C) first-K extraction with gpsimd local_scatter (reversed slots; pad slots
     fixed to first index); indices wrapped for ap_gather via PE transposes;
     SBUF-direct ap_gather of h columns; add + max-pool against the
     (host k-reversed, bf16) dlp stream.
"""
import functools
import numpy as np
import ml_dtypes

B, N, C, K = 4, 4096, 128, 32
RADIUS = 0.1
BN_EPS = 1e-5
R2 = RADIUS * RADIUS
NCHUNK = 128
NCH = N // NCHUNK          # 32
QPC = N // 2               # 2048 queries per core
QT = 128                   # queries per tile
NQT = QPC // QT            # 16
QB = 512                   # query block (dist phase)
NQB = QPC // QB            # 4
NCORES = 8
SIG_SCALE = 1e9            # saturated sigmoid: v = sig(SIG_SCALE*(R2 - d2))

bf16 = ml_dtypes.bfloat16


def _bf(x):
    return np.asarray(x, dtype=bf16).astype(np.float32)


def _split3(x):
    h = _bf(x)
    m = _bf(x - h)
    l = _bf(x - h - m)
    return h, m, l


def _build_AB(p_b, q_lo, q_hi):
    """A [24, N] (candidate side, -2 scale folded), Bm [24, Q] (query side), bf16."""
    x = p_b.astype(np.float32)
    pn = np.sum(x * x, axis=1, dtype=np.float32)
    sp = [_split3(x[:, d]) for d in range(3)]
    sp2 = [tuple(-2.0 * t for t in sp[d]) for d in range(3)]
    spn = _split3(pn)
    PAIRS = [(0, 0), (0, 1), (1, 0), (0, 2), (2, 0), (1, 1)]
    A_rows, B_rows = [], []
    for d in range(3):
        for (s, s2) in PAIRS:
            A_rows.append(sp2[d][s])
            B_rows.append(sp[d][s2][q_lo:q_hi])
    for s in range(3):
        A_rows.append(spn[s])
        B_rows.append(np.ones(q_hi - q_lo, np.float32))
    for s in range(3):
        A_rows.append(np.ones(N, np.float32))
        B_rows.append(spn[s][q_lo:q_hi])
    return (np.stack(A_rows).astype(bf16), np.stack(B_rows).astype(bf16))


def _static_tables():
    # triW[j, j'] = -1 for j<j', 32 at diag: psum = 32*v[j'] - rank_local_excl
    triW = np.zeros((NCHUNK, NCHUNK), np.float32)
    for jl in range(NCHUNK):
        triW[:jl, jl] = -1.0
        triW[jl, jl] = 32.0
    iota_j = np.broadcast_to(np.arange(N, dtype=np.int16)[None, :], (128, N)).copy()
    ident = np.eye(128, dtype=np.float32)
    ones_col = np.ones((128, 1), dtype=bf16)
    return triW.astype(bf16), iota_j, ident, ones_col


def _kernel_body(tc, io):
    import concourse.bass as bass
    import concourse.mybir as mybir
    from contextlib import ExitStack

    nc = tc.nc
    dt = mybir.dt
    AF = mybir.ActivationFunctionType
    ALU = mybir.AluOpType

    with ExitStack() as ctx:
        const = ctx.enter_context(tc.tile_pool(name="const", bufs=1))
        w1t_sb = const.tile([128, 128], dt.float32, tag="w1t")
        nc.sync.dma_start(out=w1t_sb, in_=io["w1t"])
        gamma_sb = const.tile([128, 1], dt.float32, tag="gm")
        nc.sync.dma_start(out=gamma_sb, in_=io["gamma"])
        beta_sb = const.tile([128, 1], dt.float32, tag="bt")
        nc.sync.dma_start(out=beta_sb, in_=io["beta"])
        pA_sb = const.tile([24, N], dt.bfloat16, tag="pA")
        nc.sync.dma_start(out=pA_sb, in_=io["pA"])
        pB_sb = const.tile([24, QPC], dt.bfloat16, tag="pB")
        nc.sync.dma_start(out=pB_sb, in_=io["pB"])
        tri_sb = const.tile([128, 128], dt.bfloat16, tag="tri")
        nc.sync.dma_start(out=tri_sb, in_=io["tri_w"])
        iota_sb = const.tile([128, N], dt.int16, tag="iota")
        nc.sync.dma_start(out=iota_sb, in_=io["iota_j"])
        ident_sb = const.tile([128, 128], dt.float32, tag="idn")
        nc.sync.dma_start(out=ident_sb, in_=io["ident"])
        ones_sb = const.tile([128, 1], dt.bfloat16, tag="one")
        nc.sync.dma_start(out=ones_sb, in_=io["ones_col"])
        eps_t = const.tile([128, 1], dt.float32, tag="eps")
        nc.vector.memset(eps_t, BN_EPS)
        sigb_t = const.tile([128, 1], dt.float32, tag="sgb")
        nc.vector.memset(sigb_t, SIG_SCALE * R2)

        dram = ctx.enter_context(tc.tile_pool(name="dram", bufs=1, space="DRAM"))
        hT_d = dram.tile([N, 128], dt.float16, tag="hTd")

        # ---------------- Phase A: BN stats + h_n ----------------
        with ExitStack() as actx:
            bigf = actx.enter_context(tc.tile_pool(name="bigf", bufs=1))
            ps_h = actx.enter_context(tc.tile_pool(name="ps_h", bufs=4, space="PSUM"))
            stp = actx.enter_context(tc.tile_pool(name="stp", bufs=1))

            h_n = bigf.tile([128, N], dt.float32, tag="hn")
            hT = bigf.tile([128, N], dt.float16, tag="hT")
            f_sb = bigf.tile([128, B * N], dt.float32, tag="f")
            nc.sync.dma_start(out=f_sb, in_=io["f_all"])
            f_b = bigf.tile([128, N], dt.float32, tag="fb")
            nc.sync.dma_start(out=f_b, in_=io["f_b"])

            stats = stp.tile([128, B * N // 512, 6], dt.float32, tag="st")
            for i in range(B * N // 512):
                ph = ps_h.tile([128, 512], dt.float32, tag="ph")
                nc.tensor.matmul(ph, w1t_sb, f_sb[:, i * 512:(i + 1) * 512],
                                 start=True, stop=True)
                nc.vector.bn_stats(out=stats[:, i, :], in_=ph)
            mv = stp.tile([128, 2], dt.float32, tag="mv")
            nc.vector.bn_aggr(out=mv, in_=stats)
            # rstd = 1/sqrt(var+eps); gamma2 = gamma*rstd; bias2 = beta - gamma2*mean
            sqv = stp.tile([128, 1], dt.float32, tag="sq")
            nc.scalar.activation(sqv, mv[:, 1:2], AF.Sqrt, bias=eps_t, scale=1.0)
            rstd = stp.tile([128, 1], dt.float32, tag="rs")
            nc.vector.reciprocal(rstd, sqv)
            gamma2 = stp.tile([128, 1], dt.float32, tag="g2")
            nc.vector.tensor_mul(gamma2, gamma_sb, rstd)
            gm = stp.tile([128, 1], dt.float32, tag="gmn")
            nc.vector.tensor_mul(gm, gamma2, mv[:, 0:1])
            bias2 = stp.tile([128, 1], dt.float32, tag="b2")
            nc.vector.tensor_sub(bias2, beta_sb, gm)

            for i in range(N // 512):
                ph = ps_h.tile([128, 512], dt.float32, tag="ph")
                nc.tensor.matmul(ph, w1t_sb, f_b[:, i * 512:(i + 1) * 512],
                                 start=True, stop=True)
                nc.scalar.activation(h_n[:, i * 512:(i + 1) * 512], ph, AF.Relu,
                                     bias=bias2, scale=gamma2)

            # hT fp16 rows -> DRAM table for the row gather: row n = h[:, n]
            for c2 in range(N // 128):
                tp2 = ps_h.tile([128, 128], dt.float32, tag="htp")
                nc.tensor.transpose(tp2, h_n[:, c2 * 128:(c2 + 1) * 128], ident_sb)
                nc.scalar.activation(hT[:, c2 * 128:(c2 + 1) * 128], tp2, AF.Copy)
            hT_v = hT.rearrange("p (s c) -> p s c", c=128)
            hTd_v = bass.AP(tensor=hT_d.tensor, offset=hT_d.offset,
                            ap=[[128, 128], [16384, N // 128], [1, 128]])
            nc.sync.dma_start(out=hTd_v, in_=hT_v)

        # ---------------- Phase B ----------------
        spool = ctx.enter_context(tc.tile_pool(name="sp", bufs=1))
        s_t = []
        for c in range(NCH):
            sc = spool.tile([128, QPC], dt.float8e4, tag=f"s{c}")
            s_t.append(sc)
        cpool = ctx.enter_context(tc.tile_pool(name="cp", bufs=1))
        cnt_sb = cpool.tile([128, NQT * NCH], dt.float32, tag="cnt")
        ct_sb = cpool.tile([128, NQT * NCH], dt.float32, tag="ct")
        prpool = ctx.enter_context(tc.tile_pool(name="prp", bufs=2))

        # B1: distances + v in {0,1}; B2: per-(tile,chunk) counts
        with ExitStack() as dctx:
            ps_d = dctx.enter_context(tc.tile_pool(name="ps_d", bufs=6, space="PSUM"))
            ps_c = dctx.enter_context(tc.tile_pool(name="ps_c", bufs=1, space="PSUM"))

            for qb in range(NQB):
                qbs = slice(qb * QB, (qb + 1) * QB)
                for c in range(NCH):
                    psq = ps_d.tile([128, QB], dt.float32, tag="sq")
                    nc.tensor.matmul(psq, pA_sb[:, c * NCHUNK:(c + 1) * NCHUNK],
                                     pB_sb[:, qbs], start=True, stop=True)
                    nc.scalar.activation(s_t[c][:, qbs], psq, AF.Sigmoid,
                                         bias=sigb_t, scale=-SIG_SCALE)

            cnt_ps = ps_c.tile([128, NQT * NCH], dt.float32, tag="cps")
            for t in range(NQT):
                ts = slice(t * QT, (t + 1) * QT)
                for c in range(NCH):
                    nc.tensor.matmul(cnt_ps[:, t * NCH + c:t * NCH + c + 1],
                                     s_t[c][:, ts], ones_sb, start=True, stop=True)
            nc.scalar.activation(cnt_sb, cnt_ps, AF.Copy)

        # B3: batched exclusive prefix over chunks (per tile), CT = -prefix
        cnt3 = cnt_sb.rearrange("p (t c) -> p t c", c=NCH)
        pref = prpool.tile([128, NQT * NCH], dt.float32, tag="pf")
        pr3 = pref.rearrange("p (t c) -> p t c", c=NCH)
        nc.vector.memset(pr3[:, :, 0:1], 0.0)
        nc.vector.tensor_copy(pr3[:, :, 1:], cnt3[:, :, :NCH - 1])
        for sh in (1, 2, 4, 8, 16):
            pref2 = prpool.tile([128, NQT * NCH], dt.float32, tag="pf")
            p23 = pref2.rearrange("p (t c) -> p t c", c=NCH)
            nc.vector.tensor_copy(p23[:, :, :sh], pr3[:, :, :sh])
            nc.vector.tensor_add(p23[:, :, sh:], pr3[:, :, sh:],
                                 pr3[:, :, :NCH - sh])
            pref, pr3 = pref2, p23
        nc.vector.tensor_scalar(ct_sb, pref, -1.0, None, op0=ALU.mult)

        # B4: per-chunk rank matmuls + GpSimd slot evac (carry fused)
        ps_rk = ctx.enter_context(tc.tile_pool(name="ps_rk", bufs=4, space="PSUM"))
        ps_tr = ctx.enter_context(tc.tile_pool(name="ps_tr", bufs=2, space="PSUM"))
        ipool = ctx.enter_context(tc.tile_pool(name="ip", bufs=2))
        dstp = ctx.enter_context(tc.tile_pool(name="dst", bufs=2))
        iwpool = ctx.enter_context(tc.tile_pool(name="iw", bufs=1))

        idxs_tiles = []
        for t in range(NQT):
            ts = slice(t * QT, (t + 1) * QT)
            idxs_sc = ipool.tile([128, N], dt.int16, tag="isc")
            for g in range(8):           # quarters of 4 chunks -> one PSUM bank
                pr = ps_rk.tile([128, 512], dt.float32, tag="rk")
                for cc in range(4):
                    c = 4 * g + cc
                    nc.tensor.matmul(pr[:, cc * 128:(cc + 1) * 128],
                                     s_t[c][:, ts], tri_sb, start=True, stop=True)
                ct_b = bass.AP(tensor=ct_sb.tensor,
                               offset=ct_sb.offset + t * NCH + g * 4,
                               ap=[ct_sb.ap[0], [1, 4], [0, 128]])
                nc.vector.scalar_tensor_tensor(
                    out=idxs_sc.rearrange("p (c j) -> p c j", j=128)[:, g * 4:(g + 1) * 4, :],
                    in0=pr.rearrange("p (c j) -> p c j", j=128),
                    scalar=-1.0, in1=ct_b, op0=ALU.add, op1=ALU.add)
            idxs_tiles.append(idxs_sc)

        # ---------------- Phase C ----------------
        fpool = ctx.enter_context(tc.tile_pool(name="fp", bufs=2))
        apool = ctx.enter_context(tc.tile_pool(name="ap", bufs=3))
        dpool = ctx.enter_context(tc.tile_pool(name="dp", bufs=3))
        opool = ctx.enter_context(tc.tile_pool(name="op", bufs=3))

        dlp_v = io["dlp_s"]          # [128, QPC*K] bf16, k reversed on host
        out_v = io["out_o"]          # [128, QPC] bf16

        idxw_tiles = []
        for t in range(NQT):
            # first-K extraction (reversed slots); pad slots -> first index
            dst = dstp.tile([128, K], dt.int16, tag="dst")
            nc.gpsimd.local_scatter(dst, iota_sb, idxs_tiles[t],
                                    channels=128, num_elems=K, num_idxs=N)
            mask = dstp.tile([128, K], dt.int8, tag="msk")
            nc.vector.tensor_scalar(mask, dst, 0.0, None, op0=ALU.is_equal)
            nc.vector.copy_predicated(dst, mask,
                                      dst[:, K - 1:K].to_broadcast((128, K)))
            # replicate x8 into (h, rep, s16) layout, transpose into wrapped idx
            dstf = dstp.tile([128, 256], dt.float32, tag="dsf")
            dfv = dstf.rearrange("p (h r s) -> p h r s", h=2, r=8)
            dst_b = bass.AP(tensor=dst.tensor, offset=dst.offset,
                            ap=[dst.ap[0], [16, 2], [0, 8], [1, 16]])
            nc.vector.tensor_copy(dfv, dst_b)
            idxw = iwpool.tile([128, 256], dt.int16, tag=f"idxw{t}")
            iwv = idxw.rearrange("p (c two) -> p c two", two=2)
            for half in range(2):
                tps = ps_tr.tile([128, 128], dt.float32, tag="tp")
                nc.tensor.transpose(tps, dstf[:, half * 128:(half + 1) * 128],
                                    ident_sb)
                nc.scalar.activation(iwv[:, :, half], tps, AF.Copy)
            idxw_tiles.append(idxw)

        GC = 512   # indices per dma_gather call
        PB = min(1024, GC)   # pooling block
        for t in range(NQT):
            for gg in range(QT * K // GC):
                fj_nt = fpool.tile([128, GC], dt.float16, tag="fnt")
                nc.gpsimd.dma_gather(
                    fj_nt.rearrange("p (g i) -> p g i", i=128), hT_d,
                    idxw_tiles[t][:, gg * (GC // 16):(gg + 1) * (GC // 16)],
                    num_idxs=GC, num_idxs_reg=GC, elem_size=128,
                    transpose=False)
                fj = fpool.tile([128, GC], dt.float16, tag="fj")
                eng = nc.sync if gg % 2 == 0 else nc.scalar
                eng.dma_start(out=fj.rearrange("p (g i) -> p g i", i=128),
                              in_=fj_nt, transpose=True)
                for bb in range(GC // PB):
                    m0 = t * QT * K + gg * GC + bb * PB
                    dlp_t = dpool.tile([128, PB], dt.bfloat16, tag="dl")
                    nc.sync.dma_start(out=dlp_t, in_=dlp_v[:, m0:m0 + PB])
                    addt = apool.tile([128, PB], dt.bfloat16, tag="ad")
                    nc.vector.tensor_add(addt, fj[:, bb * PB:(bb + 1) * PB], dlp_t)
                    out_t = opool.tile([128, PB // K], dt.bfloat16, tag="ot")
                    nc.vector.tensor_reduce(out_t,
                                            addt.rearrange("p (q k) -> p q k", k=K),
                                            axis=mybir.AxisListType.X, op=ALU.max)
                    nc.sync.dma_start(
                        out=out_v[:, m0 // K:m0 // K + PB // K], in_=out_t)


def _compiled():
    import concourse.bass as bass
    import concourse.bacc as bacc
    import concourse.tile as tile
    import concourse.mybir as mybir

    dt = mybir.dt
    nc = bacc.Bacc("TRN2", target_bir_lowering=False, debug=False,
                   num_devices=NCORES)
    io = {}

    def din(name, shape, dtype):
        io[name] = nc.dram_tensor(name, shape, dtype, kind="ExternalInput").ap()

    din("f_all", [128, B * N], dt.float32)
    din("f_b", [128, N], dt.float32)
    din("w1t", [128, 128], dt.float32)
    din("gamma", [128, 1], dt.float32)
    din("beta", [128, 1], dt.float32)
    din("pA", [24, N], dt.bfloat16)
    din("pB", [24, QPC], dt.bfloat16)
    din("tri_w", [128, 128], dt.bfloat16)
    din("iota_j", [128, N], dt.int16)
    din("ident", [128, 128], dt.float32)
    din("ones_col", [128, 1], dt.bfloat16)
    din("dlp_s", [128, QPC * K], dt.bfloat16)
    io["out_o"] = nc.dram_tensor("out_o", [128, QPC], dt.bfloat16,
                                 kind="ExternalOutput").ap()

    with tile.TileContext(nc) as tc:
        _kernel_body(tc, io)
    nc.compile()
    return nc


def _host_prep(inputs):
    p = np.asarray(inputs["p"], np.float32)
    f = np.asarray(inputs["f"], np.float32)
    dlp = np.asarray(inputs["dlp"], np.float32)
    W1 = np.asarray(inputs["W1"], np.float32)
    gamma = np.asarray(inputs["gamma"], np.float32)
    beta = np.asarray(inputs["beta"], np.float32)

    tri_w, iota_j, ident, ones_col = _static_tables()
    f_all = np.ascontiguousarray(np.moveaxis(f, 0, 1).reshape(C, B * N))
    w1t = np.ascontiguousarray(W1.T)

    in_maps = []
    for core in range(NCORES):
        b, half = core // 2, core % 2
        q_lo, q_hi = half * QPC, (half + 1) * QPC
        pA, pB = _build_AB(p[b], q_lo, q_hi)
        dlp_s = np.ascontiguousarray(
            dlp[b, :, q_lo:q_hi, ::-1].reshape(C, QPC * K)).astype(bf16)
        in_maps.append({
            "f_all": f_all,
            "f_b": np.ascontiguousarray(f_all[:, b * N:(b + 1) * N]),
            "w1t": w1t,
            "gamma": np.ascontiguousarray(gamma.reshape(C, 1)),
            "beta": np.ascontiguousarray(beta.reshape(C, 1)),
            "pA": pA, "pB": pB,
            "tri_w": tri_w, "iota_j": iota_j,
            "ident": ident, "ones_col": ones_col,
            "dlp_s": dlp_s,
        })
    return in_maps


def run(inputs, trace=False, **kw):
    from concourse.bass_utils import run_bass_kernel_spmd
    nc = _compiled()
    in_maps = _host_prep(inputs)
    res = run_bass_kernel_spmd(nc, in_maps, core_ids=list(range(NCORES)),
                               trace=trace, **kw)
    out = np.zeros((B, C, N), np.float32)
    for core in range(NCORES):
        b, half = core // 2, core % 2
        out[b, :, half * QPC:(half + 1) * QPC] = \
            np.asarray(res.results[core]["out_o"]).astype(np.float32)
    return out, res


def kernel(**inputs) -> np.ndarray:
    out, _ = run(inputs, trace=False)
    return out


# revision 28
# speedup vs baseline: 1.1893x; 1.1893x over previous
"""Trainium2 Bass kernel for nn_DGASEncoder (PointNet++-style ball-query encoder).

Self-contained: hardcoded shapes; takes FULL inputs, shards across 8 NeuronCores
(data-parallel over (batch, N/2)), returns the FULL output.

Per-core pipeline (SPMD; cores differ only in input data), stage-major emission
so independent query-tiles pipeline across engines:
  A) conv1d+BN stats: h = W1 @ f over all B*N via PE; bn_stats/bn_aggr -> mean/var;
     h_n = relu(gamma'*h_b + beta') for this core's batch.
  B) ball query: squared distances via a 24-row bf16-split matmul (f32-exact dot);
     v = saturated sigmoid in {0,1} on ACT; per-chunk ranks via one fp-exact
     triangular matmul per chunk; per-chunk counts via 1-col matmuls into one
     PSUM bank; batched global prefix on DVE; slot ids evacuated on GpSimd with
     the cross-chunk carry fused in (PSUM + CT -> int16).
  C) first-K extraction with gpsimd local_scatter (reversed slots; pad slots
     fixed to first index); indices wrapped for ap_gather via PE transposes;
     SBUF-direct ap_gather of h columns; add + max-pool against the
     (host k-reversed, bf16) dlp stream.
"""
import functools
import numpy as np
import ml_dtypes

B, N, C, K = 4, 4096, 128, 32
RADIUS = 0.1
BN_EPS = 1e-5
R2 = RADIUS * RADIUS
NCHUNK = 128
NCH = N // NCHUNK          # 32
QPC = N // 2               # 2048 queries per core
QT = 128                   # queries per tile
NQT = QPC // QT            # 16
QB = 512                   # query block (dist phase)
NQB = QPC // QB            # 4
NCORES = 8
SIG_SCALE = 1e9            # saturated sigmoid: v = sig(SIG_SCALE*(R2 - d2))

bf16 = ml_dtypes.bfloat16


def _bf(x):
    return np.asarray(x, dtype=bf16).astype(np.float32)


def _split3(x):
    h = _bf(x)
    m = _bf(x - h)
    l = _bf(x - h - m)
    return h, m, l


def _build_AB(p_b, q_lo, q_hi):
    """A [24, N] (candidate side, -2 scale folded), Bm [24, Q] (query side), bf16."""
    x = p_b.astype(np.float32)
    pn = np.sum(x * x, axis=1, dtype=np.float32)
    sp = [_split3(x[:, d]) for d in range(3)]
    sp2 = [tuple(-2.0 * t for t in sp[d]) for d in range(3)]
    spn = _split3(pn)
    PAIRS = [(0, 0), (0, 1), (1, 0), (0, 2), (2, 0), (1, 1)]
    A_rows, B_rows = [], []
    for d in range(3):
        for (s, s2) in PAIRS:
            A_rows.append(sp2[d][s])
            B_rows.append(sp[d][s2][q_lo:q_hi])
    for s in range(3):
        A_rows.append(spn[s])
        B_rows.append(np.ones(q_hi - q_lo, np.float32))
    for s in range(3):
        A_rows.append(np.ones(N, np.float32))
        B_rows.append(spn[s][q_lo:q_hi])
    return (np.stack(A_rows).astype(bf16), np.stack(B_rows).astype(bf16))


def _static_tables():
    # triW[j, j'] = -1 for j<j', 32 at diag: psum = 32*v[j'] - rank_local_excl
    triW = np.zeros((NCHUNK, NCHUNK), np.float32)
    for jl in range(NCHUNK):
        triW[:jl, jl] = -1.0
        triW[jl, jl] = 32.0
    iota_j = np.broadcast_to(np.arange(N, dtype=np.int16)[None, :], (128, N)).copy()
    ident = np.eye(128, dtype=np.float32)
    ones_col = np.ones((128, 1), dtype=bf16)
    return triW.astype(bf16), iota_j, ident, ones_col


def _kernel_body(tc, io):
    import concourse.bass as bass
    import concourse.mybir as mybir
    from contextlib import ExitStack

    nc = tc.nc
    dt = mybir.dt
    AF = mybir.ActivationFunctionType
    ALU = mybir.AluOpType

    with ExitStack() as ctx:
        const = ctx.enter_context(tc.tile_pool(name="const", bufs=1))
        w1t_sb = const.tile([128, 128], dt.float32, tag="w1t")
        nc.sync.dma_start(out=w1t_sb, in_=io["w1t"])
        gamma_sb = const.tile([128, 1], dt.float32, tag="gm")
        nc.sync.dma_start(out=gamma_sb, in_=io["gamma"])
        beta_sb = const.tile([128, 1], dt.float32, tag="bt")
        nc.sync.dma_start(out=beta_sb, in_=io["beta"])
        pA_sb = const.tile([24, N], dt.bfloat16, tag="pA")
        nc.sync.dma_start(out=pA_sb, in_=io["pA"])
        pB_sb = const.tile([24, QPC], dt.bfloat16, tag="pB")
        nc.sync.dma_start(out=pB_sb, in_=io["pB"])
        tri_sb = const.tile([128, 128], dt.bfloat16, tag="tri")
        nc.sync.dma_start(out=tri_sb, in_=io["tri_w"])
        iota_sb = const.tile([128, N], dt.int16, tag="iota")
        nc.sync.dma_start(out=iota_sb, in_=io["iota_j"])
        ident_sb = const.tile([128, 128], dt.float32, tag="idn")
        nc.sync.dma_start(out=ident_sb, in_=io["ident"])
        ones_sb = const.tile([128, 1], dt.bfloat16, tag="one")
        nc.sync.dma_start(out=ones_sb, in_=io["ones_col"])
        eps_t = const.tile([128, 1], dt.float32, tag="eps")
        nc.vector.memset(eps_t, BN_EPS)
        sigb_t = const.tile([128, 1], dt.float32, tag="sgb")
        nc.vector.memset(sigb_t, SIG_SCALE * R2)

        dram = ctx.enter_context(tc.tile_pool(name="dram", bufs=1, space="DRAM"))
        hT_d = dram.tile([N, 128], dt.float16, tag="hTd")

        # ---------------- Phase A: BN stats + h_n ----------------
        with ExitStack() as actx:
            bigf = actx.enter_context(tc.tile_pool(name="bigf", bufs=1))
            ps_h = actx.enter_context(tc.tile_pool(name="ps_h", bufs=4, space="PSUM"))
            stp = actx.enter_context(tc.tile_pool(name="stp", bufs=1))

            h_n = bigf.tile([128, N], dt.float32, tag="hn")
            hT = bigf.tile([128, N], dt.float16, tag="hT")
            f_sb = bigf.tile([128, B * N], dt.float32, tag="f")
            nc.sync.dma_start(out=f_sb, in_=io["f_all"])
            f_b = bigf.tile([128, N], dt.float32, tag="fb")
            nc.sync.dma_start(out=f_b, in_=io["f_b"])

            stats = stp.tile([128, B * N // 512, 6], dt.float32, tag="st")
            for i in range(B * N // 512):
                ph = ps_h.tile([128, 512], dt.float32, tag="ph")
                nc.tensor.matmul(ph, w1t_sb, f_sb[:, i * 512:(i + 1) * 512],
                                 start=True, stop=True)
                nc.vector.bn_stats(out=stats[:, i, :], in_=ph)
            mv = stp.tile([128, 2], dt.float32, tag="mv")
            nc.vector.bn_aggr(out=mv, in_=stats)
            # rstd = 1/sqrt(var+eps); gamma2 = gamma*rstd; bias2 = beta - gamma2*mean
            sqv = stp.tile([128, 1], dt.float32, tag="sq")
            nc.scalar.activation(sqv, mv[:, 1:2], AF.Sqrt, bias=eps_t, scale=1.0)
            rstd = stp.tile([128, 1], dt.float32, tag="rs")
            nc.vector.reciprocal(rstd, sqv)
            gamma2 = stp.tile([128, 1], dt.float32, tag="g2")
            nc.vector.tensor_mul(gamma2, gamma_sb, rstd)
            gm = stp.tile([128, 1], dt.float32, tag="gmn")
            nc.vector.tensor_mul(gm, gamma2, mv[:, 0:1])
            bias2 = stp.tile([128, 1], dt.float32, tag="b2")
            nc.vector.tensor_sub(bias2, beta_sb, gm)

            for i in range(N // 512):
                ph = ps_h.tile([128, 512], dt.float32, tag="ph")
                nc.tensor.matmul(ph, w1t_sb, f_b[:, i * 512:(i + 1) * 512],
                                 start=True, stop=True)
                nc.scalar.activation(h_n[:, i * 512:(i + 1) * 512], ph, AF.Relu,
                                     bias=bias2, scale=gamma2)

            # hT fp16 rows -> DRAM table for the row gather: row n = h[:, n]
            for c2 in range(N // 128):
                tp2 = ps_h.tile([128, 128], dt.float32, tag="htp")
                nc.tensor.transpose(tp2, h_n[:, c2 * 128:(c2 + 1) * 128], ident_sb)
                nc.scalar.activation(hT[:, c2 * 128:(c2 + 1) * 128], tp2, AF.Copy)
            hT_v = hT.rearrange("p (s c) -> p s c", c=128)
            hTd_v = bass.AP(tensor=hT_d.tensor, offset=hT_d.offset,
                            ap=[[128, 128], [16384, N // 128], [1, 128]])
            nc.sync.dma_start(out=hTd_v, in_=hT_v)

        # ---------------- Phase B + C: block-pipelined ----------------
        spool = ctx.enter_context(tc.tile_pool(name="sp", bufs=1))
        s_t = []
        for c in range(NCH):
            sc = spool.tile([128, QPC], dt.float8e4, tag=f"s{c}")
            s_t.append(sc)
        cpool = ctx.enter_context(tc.tile_pool(name="cp", bufs=1))
        cnt_sb = cpool.tile([128, NQT * NCH], dt.float32, tag="cnt")
        ct_sb = cpool.tile([128, NQT * NCH], dt.float32, tag="ct")
        prpool = ctx.enter_context(tc.tile_pool(name="prp", bufs=2))

        ps_d = ctx.enter_context(tc.tile_pool(name="ps_d", bufs=3, space="PSUM"))
        ps_c = ctx.enter_context(tc.tile_pool(name="ps_c", bufs=1, space="PSUM"))
        ps_rk = ctx.enter_context(tc.tile_pool(name="ps_rk", bufs=3, space="PSUM"))
        ps_tr = ctx.enter_context(tc.tile_pool(name="ps_tr", bufs=1, space="PSUM"))
        ipool = ctx.enter_context(tc.tile_pool(name="ip", bufs=2))
        dstp = ctx.enter_context(tc.tile_pool(name="dst", bufs=4))
        fpool = ctx.enter_context(tc.tile_pool(name="fp", bufs=2))
        apool = ctx.enter_context(tc.tile_pool(name="ap", bufs=2))
        dpool = ctx.enter_context(tc.tile_pool(name="dp", bufs=3))
        opool = ctx.enter_context(tc.tile_pool(name="op", bufs=3))

        dlp_v = io["dlp_s"]          # [128, QPC*K] bf16, k-major + k-reversed
        out_v = io["out_o"]          # [128, QPC] bf16
        cnt_ps = ps_c.tile([128, NQT * NCH], dt.float32, tag="cps")
        GC = 512                     # indices per dma_gather call

        for b in range(NQB):
            qbs = slice(b * QB, (b + 1) * QB)
            # B1: distances + v in {0,1}
            for c in range(NCH):
                psq = ps_d.tile([128, QB], dt.float32, tag="sq")
                nc.tensor.matmul(psq, pA_sb[:, c * NCHUNK:(c + 1) * NCHUNK],
                                 pB_sb[:, qbs], start=True, stop=True)
                nc.scalar.activation(s_t[c][:, qbs], psq, AF.Sigmoid,
                                     bias=sigb_t, scale=-SIG_SCALE)
            # B2: per-(tile,chunk) valid counts
            for tt in range(4):
                t = b * 4 + tt
                ts = slice(t * QT, (t + 1) * QT)
                for c in range(NCH):
                    nc.tensor.matmul(cnt_ps[:, t * NCH + c:t * NCH + c + 1],
                                     s_t[c][:, ts], ones_sb, start=True, stop=True)
            bsl = slice(b * 4 * NCH, (b + 1) * 4 * NCH)
            nc.scalar.activation(cnt_sb[:, bsl], cnt_ps[:, bsl], AF.Copy)
            # B3: batched exclusive prefix over chunks, CT = -prefix
            cnt3 = cnt_sb[:, bsl].rearrange("p (t c) -> p t c", c=NCH)
            pref = prpool.tile([128, 4 * NCH], dt.float32, tag="pf")
            pr3 = pref.rearrange("p (t c) -> p t c", c=NCH)
            nc.vector.memset(pr3[:, :, 0:1], 0.0)
            nc.vector.tensor_copy(pr3[:, :, 1:], cnt3[:, :, :NCH - 1])
            for sh in (1, 2, 4, 8, 16):
                pref2 = prpool.tile([128, 4 * NCH], dt.float32, tag="pf")
                p23 = pref2.rearrange("p (t c) -> p t c", c=NCH)
                nc.vector.tensor_copy(p23[:, :, :sh], pr3[:, :, :sh])
                nc.vector.tensor_add(p23[:, :, sh:], pr3[:, :, sh:],
                                     pr3[:, :, :NCH - sh])
                pref, pr3 = pref2, p23
            nc.vector.tensor_scalar(ct_sb[:, bsl], pref, -1.0, None, op0=ALU.mult)

            # B4: rank matmuls + slot evac (carry fused); C1: scatter + dst32
            idxs_blk = []
            for tt in range(4):
                t = b * 4 + tt
                ts = slice(t * QT, (t + 1) * QT)
                idxs_sc = ipool.tile([128, N], dt.int16, tag="isc")
                for g in range(8):
                    pr = ps_rk.tile([128, 512], dt.float32, tag="rk")
                    for cc in range(4):
                        c = 4 * g + cc
                        nc.tensor.matmul(pr[:, cc * 128:(cc + 1) * 128],
                                         s_t[c][:, ts], tri_sb,
                                         start=True, stop=True)
                    ct_b = bass.AP(tensor=ct_sb.tensor,
                                   offset=ct_sb.offset + t * NCH + g * 4,
                                   ap=[ct_sb.ap[0], [1, 4], [0, 128]])
                    nc.vector.scalar_tensor_tensor(
                        out=idxs_sc.rearrange("p (c j) -> p c j", j=128)[:, g * 4:(g + 1) * 4, :],
                        in0=pr.rearrange("p (c j) -> p c j", j=128),
                        scalar=-1.0, in1=ct_b, op0=ALU.add, op1=ALU.add)
                idxs_blk.append(idxs_sc)

            dst_blk = []
            for tt in range(4):
                dst = dstp.tile([128, K], dt.int16, tag="dst")
                nc.gpsimd.local_scatter(dst, iota_sb, idxs_blk[tt],
                                        channels=128, num_elems=K, num_idxs=N)
                dst_blk.append(dst)

            idxw_blk = []
            for tt in range(4):
                dst = dst_blk[tt]
                mask = dstp.tile([128, K], dt.int8, tag="msk")
                nc.vector.tensor_scalar(mask, dst, 0.0, None, op0=ALU.is_equal)
                nc.vector.copy_predicated(dst, mask,
                                          dst[:, K - 1:K].to_broadcast((128, K)))
                # replicate x8 into (h, rep, s16) layout, transpose to wrap
                dstf = dstp.tile([128, 256], dt.float32, tag="dsf")
                dfv = dstf.rearrange("p (h r s) -> p h r s", h=2, r=8)
                dst_b = bass.AP(tensor=dst.tensor, offset=dst.offset,
                                ap=[dst.ap[0], [16, 2], [0, 8], [1, 16]])
                nc.vector.tensor_copy(dfv, dst_b)
                idxw = dstp.tile([128, 256], dt.int16, tag="idxw")
                iwv = idxw.rearrange("p (c two) -> p c two", two=2)
                for half in range(2):
                    tps = ps_tr.tile([128, 128], dt.float32, tag="tp")
                    nc.tensor.transpose(tps, dstf[:, half * 128:(half + 1) * 128],
                                        ident_sb)
                    nc.scalar.activation(iwv[:, :, half], tps, AF.Copy)
                idxw_blk.append(idxw)

            # C2: gathers (token rows) + XBAR transpose + pooling
            for tt in range(4):
                t = b * 4 + tt
                fj_nt = fpool.tile([128, QT * K], dt.float16, tag="fnt")
                fv = fj_nt.rearrange("p (g i) -> p g i", i=128)
                for gc in range(QT * K // GC):
                    nc.gpsimd.dma_gather(
                        fv[:, gc * (GC // 128):(gc + 1) * (GC // 128), :], hT_d,
                        idxw_blk[tt][:, gc * (GC // 16):(gc + 1) * (GC // 16)],
                        num_idxs=GC, num_idxs_reg=GC, elem_size=128,
                        transpose=False)
                fj = fpool.tile([128, QT * K], dt.float16, tag="fj")
                eng = nc.sync if t % 2 == 0 else nc.scalar
                eng.dma_start(out=fj.rearrange("p (g i) -> p g i", i=128),
                              in_=fj_nt, transpose=True)
                dlp_t = dpool.tile([128, QT * K], dt.bfloat16, tag="dl")
                nc.sync.dma_start(out=dlp_t,
                                  in_=dlp_v[:, t * QT * K:(t + 1) * QT * K])
                addt = apool.tile([128, QT * K], dt.bfloat16, tag="ad")
                nc.vector.tensor_add(addt, fj, dlp_t)
                out_t = opool.tile([128, QT], dt.bfloat16, tag="ot")
                nc.vector.tensor_reduce(out_t,
                                        addt.rearrange("p (q k) -> p q k", k=K),
                                        axis=mybir.AxisListType.X, op=ALU.max)
                nc.sync.dma_start(out=out_v[:, t * QT:(t + 1) * QT], in_=out_t)


def _compiled():
    import concourse.bass as bass
    import concourse.bacc as bacc
    import concourse.tile as tile
    import concourse.mybir as mybir

    dt = mybir.dt
    nc = bacc.Bacc("TRN2", target_bir_lowering=False, debug=False,
                   num_devices=NCORES)
    io = {}

    def din(name, shape, dtype):
        io[name] = nc.dram_tensor(name, shape, dtype, kind="ExternalInput").ap()

    din("f_all", [128, B * N], dt.float32)
    din("f_b", [128, N], dt.float32)
    din("w1t", [128, 128], dt.float32)
    din("gamma", [128, 1], dt.float32)
    din("beta", [128, 1], dt.float32)
    din("pA", [24, N], dt.bfloat16)
    din("pB", [24, QPC], dt.bfloat16)
    din("tri_w", [128, 128], dt.bfloat16)
    din("iota_j", [128, N], dt.int16)
    din("ident", [128, 128], dt.float32)
    din("ones_col", [128, 1], dt.bfloat16)
    din("dlp_s", [128, QPC * K], dt.bfloat16)
    io["out_o"] = nc.dram_tensor("out_o", [128, QPC], dt.bfloat16,
                                 kind="ExternalOutput").ap()

    with tile.TileContext(nc) as tc:
        _kernel_body(tc, io)
    nc.compile()
    return nc


def _host_prep(inputs):
    p = np.asarray(inputs["p"], np.float32)
    f = np.asarray(inputs["f"], np.float32)
    dlp = np.asarray(inputs["dlp"], np.float32)
    W1 = np.asarray(inputs["W1"], np.float32)
    gamma = np.asarray(inputs["gamma"], np.float32)
    beta = np.asarray(inputs["beta"], np.float32)

    tri_w, iota_j, ident, ones_col = _static_tables()
    f_all = np.ascontiguousarray(np.moveaxis(f, 0, 1).reshape(C, B * N))
    w1t = np.ascontiguousarray(W1.T)

    in_maps = []
    for core in range(NCORES):
        b, half = core // 2, core % 2
        q_lo, q_hi = half * QPC, (half + 1) * QPC
        pA, pB = _build_AB(p[b], q_lo, q_hi)
        dlp_s = np.ascontiguousarray(
            dlp[b, :, q_lo:q_hi, ::-1].reshape(C, QPC * K)).astype(bf16)
        in_maps.append({
            "f_all": f_all,
            "f_b": np.ascontiguousarray(f_all[:, b * N:(b + 1) * N]),
            "w1t": w1t,
            "gamma": np.ascontiguousarray(gamma.reshape(C, 1)),
            "beta": np.ascontiguousarray(beta.reshape(C, 1)),
            "pA": pA, "pB": pB,
            "tri_w": tri_w, "iota_j": iota_j,
            "ident": ident, "ones_col": ones_col,
            "dlp_s": dlp_s,
        })
    return in_maps


def run(inputs, trace=False, **kw):
    from concourse.bass_utils import run_bass_kernel_spmd
    nc = _compiled()
    in_maps = _host_prep(inputs)
    res = run_bass_kernel_spmd(nc, in_maps, core_ids=list(range(NCORES)),
                               trace=trace, **kw)
    out = np.zeros((B, C, N), np.float32)
    for core in range(NCORES):
        b, half = core // 2, core % 2
        out[b, :, half * QPC:(half + 1) * QPC] = \
            np.asarray(res.results[core]["out_o"]).astype(np.float32)
    return out, res


def kernel(**inputs) -> np.ndarray:
    out, _ = run(inputs, trace=False)
    return out


# revision 29
# speedup vs baseline: 1.8332x; 1.5414x over previous
"""Trainium2 Bass kernel for nn_DGASEncoder (PointNet++-style ball-query encoder).

Self-contained: hardcoded shapes; takes FULL inputs, shards across 8 NeuronCores
(data-parallel over (batch, N/2)), returns the FULL output.

Per-core pipeline (SPMD; cores differ only in input data), stage-major emission
so independent query-tiles pipeline across engines:
  A) conv1d+BN stats: h = W1 @ f over all B*N via PE; bn_stats/bn_aggr -> mean/var;
     h_n = relu(gamma'*h_b + beta') for this core's batch.
  B) ball query: squared distances via a 24-row bf16-split matmul (f32-exact dot);
     v = saturated sigmoid in {0,1} on ACT; per-chunk ranks via one fp-exact
     triangular matmul per chunk; per-chunk counts via 1-col matmuls into one
     PSUM bank; batched global prefix on DVE; slot ids evacuated on GpSimd with
     the cross-chunk carry fused in (PSUM + CT -> int16).
  C) first-K extraction with gpsimd local_scatter (reversed slots; pad slots
     fixed to first index); indices wrapped for ap_gather via PE transposes;
     SBUF-direct ap_gather of h columns; add + max-pool against the
     (host k-reversed, bf16) dlp stream.
"""
import functools
import numpy as np
import ml_dtypes

B, N, C, K = 4, 4096, 128, 32
RADIUS = 0.1
BN_EPS = 1e-5
R2 = RADIUS * RADIUS
NCHUNK = 128
NCH = N // NCHUNK          # 32
QPC = N // 2               # 2048 queries per core
QT = 128                   # queries per tile
NQT = QPC // QT            # 16
QB = 512                   # query block (dist phase)
NQB = QPC // QB            # 4
NCORES = 8
SIG_SCALE = 1e9            # saturated sigmoid: v = sig(SIG_SCALE*(R2 - d2))

bf16 = ml_dtypes.bfloat16


def _bf(x):
    return np.asarray(x, dtype=bf16).astype(np.float32)


def _split3(x):
    h = _bf(x)
    m = _bf(x - h)
    l = _bf(x - h - m)
    return h, m, l


def _build_AB(p_b, q_lo, q_hi):
    """A [24, N] (candidate side, -2 scale folded), Bm [24, Q] (query side), bf16."""
    x = p_b.astype(np.float32)
    pn = np.sum(x * x, axis=1, dtype=np.float32)
    sp = [_split3(x[:, d]) for d in range(3)]
    sp2 = [tuple(-2.0 * t for t in sp[d]) for d in range(3)]
    spn = _split3(pn)
    PAIRS = [(0, 0), (0, 1), (1, 0), (0, 2), (2, 0), (1, 1)]
    A_rows, B_rows = [], []
    for d in range(3):
        for (s, s2) in PAIRS:
            A_rows.append(sp2[d][s])
            B_rows.append(sp[d][s2][q_lo:q_hi])
    for s in range(3):
        A_rows.append(spn[s])
        B_rows.append(np.ones(q_hi - q_lo, np.float32))
    for s in range(3):
        A_rows.append(np.ones(N, np.float32))
        B_rows.append(spn[s][q_lo:q_hi])
    return (np.stack(A_rows).astype(bf16), np.stack(B_rows).astype(bf16))


def _static_tables():
    # triW[j, j'] = -1 for j<j', 32 at diag: psum = 32*v[j'] - rank_local_excl
    triW = np.zeros((NCHUNK, NCHUNK), np.float32)
    for jl in range(NCHUNK):
        triW[:jl, jl] = -1.0
        triW[jl, jl] = 32.0
    iota_j = np.broadcast_to(np.arange(N, dtype=np.int16)[None, :], (128, N)).copy()
    ident = np.eye(128, dtype=np.float32)
    ones_col = np.ones((128, 1), dtype=bf16)
    return triW.astype(bf16), iota_j, ident, ones_col


def _kernel_body(tc, io):
    import concourse.bass as bass
    import concourse.mybir as mybir
    from contextlib import ExitStack

    nc = tc.nc
    dt = mybir.dt
    AF = mybir.ActivationFunctionType
    ALU = mybir.AluOpType

    with ExitStack() as ctx:
        const = ctx.enter_context(tc.tile_pool(name="const", bufs=1))
        w1t_sb = const.tile([128, 128], dt.float32, tag="w1t")
        nc.sync.dma_start(out=w1t_sb, in_=io["w1t"])
        gamma_sb = const.tile([128, 1], dt.float32, tag="gm")
        nc.sync.dma_start(out=gamma_sb, in_=io["gamma"])
        beta_sb = const.tile([128, 1], dt.float32, tag="bt")
        nc.sync.dma_start(out=beta_sb, in_=io["beta"])
        pA_sb = const.tile([24, N], dt.bfloat16, tag="pA")
        nc.sync.dma_start(out=pA_sb, in_=io["pA"])
        pB_sb = const.tile([24, QPC], dt.bfloat16, tag="pB")
        nc.sync.dma_start(out=pB_sb, in_=io["pB"])
        tri_sb = const.tile([128, 128], dt.bfloat16, tag="tri")
        nc.sync.dma_start(out=tri_sb, in_=io["tri_w"])
        iota_sb = const.tile([128, N], dt.int16, tag="iota")
        nc.sync.dma_start(out=iota_sb, in_=io["iota_j"])
        ident_sb = const.tile([128, 128], dt.float32, tag="idn")
        nc.sync.dma_start(out=ident_sb, in_=io["ident"])
        ones_sb = const.tile([128, 1], dt.bfloat16, tag="one")
        nc.sync.dma_start(out=ones_sb, in_=io["ones_col"])
        eps_t = const.tile([128, 1], dt.float32, tag="eps")
        nc.vector.memset(eps_t, BN_EPS)
        sigb_t = const.tile([128, 1], dt.float32, tag="sgb")
        nc.vector.memset(sigb_t, SIG_SCALE * R2)

        dram = ctx.enter_context(tc.tile_pool(name="dram", bufs=1, space="DRAM"))
        hT_d = dram.tile([N, 128], dt.float16, tag="hTd")

        # ---------------- Phase A: BN stats + h_n ----------------
        with ExitStack() as actx:
            bigf = actx.enter_context(tc.tile_pool(name="bigf", bufs=1))
            ps_h = actx.enter_context(tc.tile_pool(name="ps_h", bufs=4, space="PSUM"))
            stp = actx.enter_context(tc.tile_pool(name="stp", bufs=1))

            h_n = bigf.tile([128, N], dt.float32, tag="hn")
            hT = bigf.tile([128, N], dt.float16, tag="hT")
            f_sb = bigf.tile([128, B * N], dt.float32, tag="f")
            nc.sync.dma_start(out=f_sb, in_=io["f_all"])
            f_b = bigf.tile([128, N], dt.float32, tag="fb")
            nc.sync.dma_start(out=f_b, in_=io["f_b"])

            stats = stp.tile([128, B * N // 512, 6], dt.float32, tag="st")
            for i in range(B * N // 512):
                ph = ps_h.tile([128, 512], dt.float32, tag="ph")
                nc.tensor.matmul(ph, w1t_sb, f_sb[:, i * 512:(i + 1) * 512],
                                 start=True, stop=True)
                nc.vector.bn_stats(out=stats[:, i, :], in_=ph)
            mv = stp.tile([128, 2], dt.float32, tag="mv")
            nc.vector.bn_aggr(out=mv, in_=stats)
            # rstd = 1/sqrt(var+eps); gamma2 = gamma*rstd; bias2 = beta - gamma2*mean
            sqv = stp.tile([128, 1], dt.float32, tag="sq")
            nc.scalar.activation(sqv, mv[:, 1:2], AF.Sqrt, bias=eps_t, scale=1.0)
            rstd = stp.tile([128, 1], dt.float32, tag="rs")
            nc.vector.reciprocal(rstd, sqv)
            gamma2 = stp.tile([128, 1], dt.float32, tag="g2")
            nc.vector.tensor_mul(gamma2, gamma_sb, rstd)
            gm = stp.tile([128, 1], dt.float32, tag="gmn")
            nc.vector.tensor_mul(gm, gamma2, mv[:, 0:1])
            bias2 = stp.tile([128, 1], dt.float32, tag="b2")
            nc.vector.tensor_sub(bias2, beta_sb, gm)

            for i in range(N // 512):
                ph = ps_h.tile([128, 512], dt.float32, tag="ph")
                nc.tensor.matmul(ph, w1t_sb, f_b[:, i * 512:(i + 1) * 512],
                                 start=True, stop=True)
                nc.scalar.activation(h_n[:, i * 512:(i + 1) * 512], ph, AF.Relu,
                                     bias=bias2, scale=gamma2)

            # hT fp16 rows -> DRAM table for the row gather: row n = h[:, n]
            for c2 in range(N // 128):
                tp2 = ps_h.tile([128, 128], dt.float32, tag="htp")
                nc.tensor.transpose(tp2, h_n[:, c2 * 128:(c2 + 1) * 128], ident_sb)
                nc.scalar.activation(hT[:, c2 * 128:(c2 + 1) * 128], tp2, AF.Copy)
            hT_v = hT.rearrange("p (s c) -> p s c", c=128)
            hTd_v = bass.AP(tensor=hT_d.tensor, offset=hT_d.offset,
                            ap=[[128, 128], [16384, N // 128], [1, 128]])
            nc.sync.dma_start(out=hTd_v, in_=hT_v)

        # ---------------- Phase B + C: block-pipelined ----------------
        spool = ctx.enter_context(tc.tile_pool(name="sp", bufs=1))
        s_t = []
        for c in range(NCH):
            sc = spool.tile([128, QPC], dt.float8e4, tag=f"s{c}")
            s_t.append(sc)
        cpool = ctx.enter_context(tc.tile_pool(name="cp", bufs=1))
        cnt_sb = cpool.tile([128, NQT * NCH], dt.float32, tag="cnt")
        ct_sb = cpool.tile([128, NQT * NCH], dt.float32, tag="ct")
        prpool = ctx.enter_context(tc.tile_pool(name="prp", bufs=2))

        ps_d = ctx.enter_context(tc.tile_pool(name="ps_d", bufs=3, space="PSUM"))
        ps_c = ctx.enter_context(tc.tile_pool(name="ps_c", bufs=1, space="PSUM"))
        ps_rk = ctx.enter_context(tc.tile_pool(name="ps_rk", bufs=3, space="PSUM"))
        ps_tr = ctx.enter_context(tc.tile_pool(name="ps_tr", bufs=1, space="PSUM"))
        ipool = ctx.enter_context(tc.tile_pool(name="ip", bufs=2))
        dstp = ctx.enter_context(tc.tile_pool(name="dst", bufs=4))
        fpool = ctx.enter_context(tc.tile_pool(name="fp", bufs=2))
        apool = ctx.enter_context(tc.tile_pool(name="ap", bufs=2))
        dpool = ctx.enter_context(tc.tile_pool(name="dp", bufs=3))
        opool = ctx.enter_context(tc.tile_pool(name="op", bufs=3))

        dlp_v = io["dlp_s"]          # [128, QPC*K] bf16, k-major + k-reversed
        out_v = io["out_o"]          # [128, QPC] bf16
        cnt_ps = ps_c.tile([128, NQT * NCH], dt.float32, tag="cps")
        GC = 512                     # indices per dma_gather call

        for b in range(NQB):
            qbs = slice(b * QB, (b + 1) * QB)
            # B1: distances + v in {0,1}
            for c in range(NCH):
                psq = ps_d.tile([128, QB], dt.float32, tag="sq")
                nc.tensor.matmul(psq, pA_sb[:, c * NCHUNK:(c + 1) * NCHUNK],
                                 pB_sb[:, qbs], start=True, stop=True)
                nc.scalar.activation(s_t[c][:, qbs], psq, AF.Sigmoid,
                                     bias=sigb_t, scale=-SIG_SCALE)
            # B2: per-(tile,chunk) valid counts
            for tt in range(4):
                t = b * 4 + tt
                ts = slice(t * QT, (t + 1) * QT)
                for c in range(NCH):
                    nc.tensor.matmul(cnt_ps[:, t * NCH + c:t * NCH + c + 1],
                                     s_t[c][:, ts], ones_sb, start=True, stop=True)
            bsl = slice(b * 4 * NCH, (b + 1) * 4 * NCH)
            nc.scalar.activation(cnt_sb[:, bsl], cnt_ps[:, bsl], AF.Copy)
            # B3: batched exclusive prefix over chunks, CT = -prefix
            cnt3 = cnt_sb[:, bsl].rearrange("p (t c) -> p t c", c=NCH)
            pref = prpool.tile([128, 4 * NCH], dt.float32, tag="pf")
            pr3 = pref.rearrange("p (t c) -> p t c", c=NCH)
            nc.vector.memset(pr3[:, :, 0:1], 0.0)
            nc.vector.tensor_copy(pr3[:, :, 1:], cnt3[:, :, :NCH - 1])
            for sh in (1, 2, 4, 8, 16):
                pref2 = prpool.tile([128, 4 * NCH], dt.float32, tag="pf")
                p23 = pref2.rearrange("p (t c) -> p t c", c=NCH)
                nc.vector.tensor_copy(p23[:, :, :sh], pr3[:, :, :sh])
                nc.vector.tensor_add(p23[:, :, sh:], pr3[:, :, sh:],
                                     pr3[:, :, :NCH - sh])
                pref, pr3 = pref2, p23
            nc.vector.tensor_scalar(ct_sb[:, bsl], pref, -1.0, None, op0=ALU.mult)

            # B4: rank matmuls + slot evac (carry fused); C1: scatter + dst32
            idxs_blk = []
            for tt in range(4):
                t = b * 4 + tt
                ts = slice(t * QT, (t + 1) * QT)
                idxs_sc = ipool.tile([128, N], dt.int16, tag="isc")
                for g in range(8):
                    pr = ps_rk.tile([128, 512], dt.float32, tag="rk")
                    for cc in range(4):
                        c = 4 * g + cc
                        nc.tensor.matmul(pr[:, cc * 128:(cc + 1) * 128],
                                         s_t[c][:, ts], tri_sb,
                                         start=True, stop=True)
                    ct_b = bass.AP(tensor=ct_sb.tensor,
                                   offset=ct_sb.offset + t * NCH + g * 4,
                                   ap=[ct_sb.ap[0], [1, 4], [0, 128]])
                    nc.vector.scalar_tensor_tensor(
                        out=idxs_sc.rearrange("p (c j) -> p c j", j=128)[:, g * 4:(g + 1) * 4, :],
                        in0=pr.rearrange("p (c j) -> p c j", j=128),
                        scalar=-1.0, in1=ct_b, op0=ALU.add, op1=ALU.add)
                idxs_blk.append(idxs_sc)

            dst_blk = []
            for tt in range(4):
                dst = dstp.tile([128, K], dt.int16, tag="dst")
                nc.gpsimd.local_scatter(dst, iota_sb, idxs_blk[tt],
                                        channels=128, num_elems=K, num_idxs=N)
                dst_blk.append(dst)

            idxw_blk = []
            for tt in range(4):
                dst = dst_blk[tt]
                mask = dstp.tile([128, K], dt.int8, tag="msk")
                nc.vector.tensor_scalar(mask, dst, 0.0, None, op0=ALU.is_equal)
                nc.vector.copy_predicated(dst, mask,
                                          dst[:, K - 1:K].to_broadcast((128, K)))
                # replicate x8 into (h, rep, s16) layout, transpose to wrap
                dstf = dstp.tile([128, 256], dt.float32, tag="dsf")
                dfv = dstf.rearrange("p (h r s) -> p h r s", h=2, r=8)
                dst_b = bass.AP(tensor=dst.tensor, offset=dst.offset,
                                ap=[dst.ap[0], [16, 2], [0, 8], [1, 16]])
                nc.vector.tensor_copy(dfv, dst_b)
                idxw = dstp.tile([128, 256], dt.int16, tag="idxw")
                iwv = idxw.rearrange("p (c two) -> p c two", two=2)
                for half in range(2):
                    tps = ps_tr.tile([128, 128], dt.float32, tag="tp")
                    nc.tensor.transpose(tps, dstf[:, half * 128:(half + 1) * 128],
                                        ident_sb)
                    nc.scalar.activation(iwv[:, :, half], tps, AF.Copy)
                idxw_blk.append(idxw)

            # C2: gathers (token rows) + XBAR transpose + pooling
            for tt in range(4):
                t = b * 4 + tt
                fj_nt = fpool.tile([128, QT * K], dt.float16, tag="fnt")
                fv = fj_nt.rearrange("p (g i) -> p g i", i=128)
                for gc in range(QT * K // GC):
                    nc.gpsimd.dma_gather(
                        fv[:, gc * (GC // 128):(gc + 1) * (GC // 128), :], hT_d,
                        idxw_blk[tt][:, gc * (GC // 16):(gc + 1) * (GC // 16)],
                        num_idxs=GC, num_idxs_reg=GC, elem_size=128,
                        transpose=False, queue_num=gc % 4)
                fj = fpool.tile([128, QT * K], dt.float16, tag="fj")
                eng = nc.sync if t % 2 == 0 else nc.scalar
                eng.dma_start(out=fj.rearrange("p (g i) -> p g i", i=128),
                              in_=fj_nt, transpose=True)
                dlp_t = dpool.tile([128, QT * K], dt.bfloat16, tag="dl")
                nc.sync.dma_start(out=dlp_t,
                                  in_=dlp_v[:, t * QT * K:(t + 1) * QT * K])
                addt = apool.tile([128, QT * K], dt.bfloat16, tag="ad")
                nc.vector.tensor_add(addt, fj, dlp_t)
                out_t = opool.tile([128, QT], dt.bfloat16, tag="ot")
                nc.vector.tensor_reduce(out_t,
                                        addt.rearrange("p (q k) -> p q k", k=K),
                                        axis=mybir.AxisListType.X, op=ALU.max)
                nc.sync.dma_start(out=out_v[:, t * QT:(t + 1) * QT], in_=out_t)


def _compiled():
    import concourse.bass as bass
    import concourse.bacc as bacc
    import concourse.tile as tile
    import concourse.mybir as mybir

    dt = mybir.dt
    nc = bacc.Bacc("TRN2", target_bir_lowering=False, debug=False,
                   num_devices=NCORES, num_swdge_queues=4)
    io = {}

    def din(name, shape, dtype):
        io[name] = nc.dram_tensor(name, shape, dtype, kind="ExternalInput").ap()

    din("f_all", [128, B * N], dt.float32)
    din("f_b", [128, N], dt.float32)
    din("w1t", [128, 128], dt.float32)
    din("gamma", [128, 1], dt.float32)
    din("beta", [128, 1], dt.float32)
    din("pA", [24, N], dt.bfloat16)
    din("pB", [24, QPC], dt.bfloat16)
    din("tri_w", [128, 128], dt.bfloat16)
    din("iota_j", [128, N], dt.int16)
    din("ident", [128, 128], dt.float32)
    din("ones_col", [128, 1], dt.bfloat16)
    din("dlp_s", [128, QPC * K], dt.bfloat16)
    io["out_o"] = nc.dram_tensor("out_o", [128, QPC], dt.bfloat16,
                                 kind="ExternalOutput").ap()

    with tile.TileContext(nc) as tc:
        _kernel_body(tc, io)
    nc.compile()
    return nc


def _host_prep(inputs):
    p = np.asarray(inputs["p"], np.float32)
    f = np.asarray(inputs["f"], np.float32)
    dlp = np.asarray(inputs["dlp"], np.float32)
    W1 = np.asarray(inputs["W1"], np.float32)
    gamma = np.asarray(inputs["gamma"], np.float32)
    beta = np.asarray(inputs["beta"], np.float32)

    tri_w, iota_j, ident, ones_col = _static_tables()
    f_all = np.ascontiguousarray(np.moveaxis(f, 0, 1).reshape(C, B * N))
    w1t = np.ascontiguousarray(W1.T)

    in_maps = []
    for core in range(NCORES):
        b, half = core // 2, core % 2
        q_lo, q_hi = half * QPC, (half + 1) * QPC
        pA, pB = _build_AB(p[b], q_lo, q_hi)
        dlp_s = np.ascontiguousarray(
            dlp[b, :, q_lo:q_hi, ::-1].reshape(C, QPC * K)).astype(bf16)
        in_maps.append({
            "f_all": f_all,
            "f_b": np.ascontiguousarray(f_all[:, b * N:(b + 1) * N]),
            "w1t": w1t,
            "gamma": np.ascontiguousarray(gamma.reshape(C, 1)),
            "beta": np.ascontiguousarray(beta.reshape(C, 1)),
            "pA": pA, "pB": pB,
            "tri_w": tri_w, "iota_j": iota_j,
            "ident": ident, "ones_col": ones_col,
            "dlp_s": dlp_s,
        })
    return in_maps


def run(inputs, trace=False, **kw):
    from concourse.bass_utils import run_bass_kernel_spmd
    nc = _compiled()
    in_maps = _host_prep(inputs)
    res = run_bass_kernel_spmd(nc, in_maps, core_ids=list(range(NCORES)),
                               trace=trace, **kw)
    out = np.zeros((B, C, N), np.float32)
    for core in range(NCORES):
        b, half = core // 2, core % 2
        out[b, :, half * QPC:(half + 1) * QPC] = \
            np.asarray(res.results[core]["out_o"]).astype(np.float32)
    return out, res


def kernel(**inputs) -> np.ndarray:
    out, _ = run(inputs, trace=False)
    return out
